# revision 2
# baseline (speedup 1.0000x reference)
"""GRU (hidden_size=1) kernel for Trainium2 — fp8 DoubleRow production +
hardware-linear-scan Picard iteration.  Data-parallel over batch on 8 cores.

Per core (B_loc = 256 = 2 halves x 128 partitions):

Production (PE):
  - x quantized to fp8e4 (x*16), w split hi/lo fp8 (w*64 + residual) with a
    4th negated-z row (v) so u = 1-z = sig(v-preact) comes free.
  - DoubleRow matmuls contract K=256/instr; 4 timestep-pairs accumulate into
    one PSUM bank [32, 512] (rows = pair*8 + hl*4 + gate) via zero-slotted
    weights, so the row compaction is free PSUM accumulation.
  - ACT evicts [32, 512] -> SBUF f32r; a K=32 f32r matmul against
    S = kron(I4, [I4; I4]) both transposes (gates -> batch partitions) and
    sums the hi+lo rows: out [128b, (pair, gate)] in PSUM.
  - DVE scalar_tensor_tensor evicts with fused dequant scale + per-gate
    bias into gi planes [128, (gate4, t128, half2)].

Scan (DVE/ACT): Picard iteration on the frozen-gate linear recurrence
  h' = z*h + u*n solved exactly per 32-step chunk with the hardware
  tensor_tensor_scan (state = data0*state + data1); 4 iterations converge
  far below the fp8 quantization floor.  Chunk c's iterations are emitted
  during production of blocks 4c+4..4c+7 so only the last chunk is a tail.

Final: ones-matmul over partitions -> per-(t,half) batch sums; host sums
cores/halves and divides by B.
"""

import numpy as np
import ml_dtypes

import concourse.bass as bass
import concourse.mybir as mybir
from concourse.bass_types import AP
from concourse.tile import TileContext
from concourse.bass_utils import run_bass_kernel_spmd

F32 = mybir.dt.float32
F32R = mybir.dt.float32r
FP8 = mybir.dt.float8e4
AF = mybir.ActivationFunctionType
ALU = mybir.AluOpType

N_CORES = 8
B, T, D = 2048, 128, 512
B_LOC = B // N_CORES            # 256
NBLK = 16                       # blocks
TPB = 8                         # timesteps per block (4 pairs x 2)
NPAIR = 4                       # pairs per block
CHUNK = 32                      # Picard chunk (timesteps)
NCHUNK = T // CHUNK             # 4
ITERS = 6                       # Picard iterations per chunk
XSC = 16.0                      # x quantization scale
WSC = 64.0                      # w quantization scale
DQ = 1.0 / (XSC * WSC)          # dequant at eviction
GP = 2 * T                      # gi plane pitch (t,half cols per gate)
_CACHE = {}


def _ap(tile, colofs, dims):
    """AP over `tile` at element column offset `colofs` with free dims
    [(stride, count), ...] (strides in elements)."""
    return AP(
        tensor=tile.tensor,
        offset=tile.offset + colofs,
        ap=[list(tile.ap[0])] + [[s, c] for s, c in dims],
    )


def build_nc():
    nc = bass.Bass(trn_type="TRN2")

    x8 = nc.dram_tensor("x8", [NBLK, 128, NPAIR * 2 * 2 * 512], FP8,
                        kind="ExternalInput")
    w8 = nc.dram_tensor("w8", [128, 2 * NPAIR * 2 * 32], FP8,
                        kind="ExternalInput")
    s32 = nc.dram_tensor("s32", [128, 64], F32, kind="ExternalInput")
    cst = nc.dram_tensor("cst", [128, 13], F32, kind="ExternalInput")
    out = nc.dram_tensor("out", [1, 2 * T], F32, kind="ExternalOutput")

    with TileContext(nc) as tc:
        with (
            tc.tile_pool(name="xpool", bufs=4) as xpool,
            tc.tile_pool(name="consts", bufs=1) as consts,
            tc.tile_pool(name="scan", bufs=1) as scan,
            tc.tile_pool(name="pic", bufs=2) as pic,
            tc.tile_pool(name="gia", bufs=4, space="PSUM") as gia,
            tc.tile_pool(name="gib", bufs=1, space="PSUM") as gib,
            tc.tile_pool(name="warm", bufs=1, space="PSUM") as warmp,
            tc.tile_pool(name="sump", bufs=1, space="PSUM") as sump,
        ):
            # ---- inputs ----
            x_tiles = {}

            def dma_x(blk):
                x_sb = xpool.tile([128, NPAIR, 2, 2, 512], FP8, name="x_sb")
                nc.sync.dma_start(
                    out=x_sb,
                    in_=AP(tensor=x8, offset=blk * 128 * 8192,
                           ap=[[8192, 128], [1, 8192]]),
                )
                x_tiles[blk] = x_sb

            dma_x(0)
            dma_x(1)
            dma_x(2)

            w8_sb = consts.tile([128, 2, NPAIR, 2, 32], FP8)
            nc.sync.dma_start(
                out=w8_sb,
                in_=w8[:].rearrange("p (c s i m) -> p c s i m", c=2, s=NPAIR,
                                    i=2),
            )
            s_sb = consts.tile([128, 64], F32)
            nc.sync.dma_start(out=s_sb, in_=s32[:])
            cst_sb = consts.tile([128, 13], F32)
            nc.sync.dma_start(out=cst_sb, in_=cst[:])
            bias4 = cst_sb[:, 0:4]
            w3 = cst_sb[:, 4:7]       # [w0, w1, -w1]
            h0t = cst_sb[:, 7:9]
            ones1 = cst_sb[:, 9:10]
            w2s = cst_sb[:, 10:11]
            b2s = cst_sb[:, 11:12]
            bias32 = cst_sb[0:32, 12:13]

            # ---- warm-ups: absorb const-DMA queue sems, one wait each ----
            warm_ps = warmp.tile([128, 4], F32, tag="warm")
            nc.tensor.matmul(warm_ps[0:64, :], s_sb[:], s_sb[:, 0:4],
                             start=True, stop=True)
            nc.tensor.matmul(warm_ps[0:8, :], w8_sb[:, 0, 0, 0, 0:8],
                             w8_sb[:, 0, 0, 0, 0:4], start=True, stop=True)
            nc.tensor.matmul(warm_ps[0:4, :], cst_sb[0:32, 0:4],
                             cst_sb[0:32, 0:4], start=True, stop=True)
            vwarm = consts.tile([128, 1], F32)
            nc.vector.tensor_copy(vwarm, cst_sb[:, 0:1])
            pwarm = consts.tile([128, 1], F32)
            nc.gpsimd.tensor_copy(pwarm, cst_sb[:, 0:1])
            swarm = consts.tile([128, 1], F32)
            nc.scalar.copy(swarm, cst_sb[:, 0:1])

            # ---- persistent scan state ----
            # h_sb col = (1 + t) * 2 + half;  cols 0:2 = h0
            h_sb = scan.tile([128, 2 * (T + 1)], F32)
            nc.vector.tensor_copy(h_sb[:, 0:2], h0t)
            nc.vector.memset(h_sb[:, 2:], 0.0)
            # gi planes: col = g*GP + t*2 + half
            gi_sb = scan.tile([128, 4 * GP], F32)
            # all 4 groups' transpose outputs side by side (2 PSUM banks)
            tr_ps = gib.tile([128, 256 * (NBLK // 4)], F32)
            git_tiles = [scan.tile([128, 512], F32, name=f"git{i}")
                         for i in range(2)]

            def produce_block(blk):
                if blk + 3 < NBLK and (blk + 3) not in x_tiles:
                    dma_x(blk + 3)
                x_sb = x_tiles.pop(blk)
                gi_ps = gia.tile([32, 512], F32, name="gi_ps")
                if blk >= 4:
                    # PE fence written INTO this bank: carries the WAR on the
                    # ACT git-copy that last read it (single wait), and the
                    # WAW pins it before the matmul group under the
                    # scheduler.  Output discarded by the group's start=True.
                    pg = (blk - 4) // 4
                    nc.tensor.matmul(gi_ps[0:4, 0:4],
                                     git_tiles[pg % 2][0:32, 0:4],
                                     cst_sb[0:32, 0:4], start=True, stop=True)
                for p in range(NPAIR):
                    for c in range(2):
                        nc.tensor.matmul(
                            gi_ps,
                            w8_sb[:, c, p, :, :],
                            x_sb[:, p, c, :, :],
                            start=(p == 0 and c == 0),
                            stop=(p == NPAIR - 1 and c == 1),
                            perf_mode=mybir.MatmulPerfMode.DoubleRow,
                        )
                grp, gb = blk // 4, blk % 4
                git_sb = git_tiles[grp % 2]
                nc.scalar.activation(
                    out=git_sb[32 * gb:32 * (gb + 1), :], in_=gi_ps,
                    func=AF.Identity, bias=bias32, scale=DQ)
                if gb == 3:
                    # whole group evicted: transpose 32 timesteps in 4 matmuls
                    tg0 = grp * 4 * TPB
                    for tq in range(2):
                        for half in range(2):
                            slot = tq * 2 + half
                            nc.tensor.matmul(
                                tr_ps[:, 256 * grp + 64 * slot:
                                      256 * grp + 64 * (slot + 1)],
                                git_sb[:, 256 * tq + 128 * half:
                                       256 * tq + 128 * (half + 1)],
                                s_sb[:],
                                start=True, stop=True,
                            )
                    for tq in range(2):
                        for half in range(2):
                            slot = tq * 2 + half
                            src = _ap(tr_ps, 256 * grp + 64 * slot,
                                      [(4, 16), (1, 4)])
                            dst = _ap(gi_sb, (tg0 + tq) * 2 + half,
                                      [(4, 16), (GP, 4)])
                            nc.scalar.copy(dst, src)

            def picard_iter(cidx, pool_eng=False, sfx=""):
                # pool_eng: run hw6/acc6/hw2/rn on GPSIMD so overlapped
                # chunks do not contend with DVE's eviction/scan work; the
                # tail chunk stays all-DVE for minimum chain latency.
                e1 = nc.vector
                e2 = nc.gpsimd if pool_eng else nc.vector
                t0 = cidx * CHUNK
                tc2 = 2 * CHUNK
                hprev2 = _ap(h_sb, t0 * 2, [(1, tc2)])
                hprev3 = _ap(h_sb, t0 * 2, [(0, 3), (1, tc2)])
                w3v = _ap(w3, 0, [(1, 3), (0, tc2)])
                hw6 = pic.tile([128, 3 * tc2], F32, tag="hw6" + sfx, name="hw6")
                hw6v = _ap(hw6, 0, [(tc2, 3), (1, tc2)])
                e1.tensor_tensor(hw6v, hprev3, w3v, op=ALU.mult)
                acc6 = pic.tile([128, 3 * tc2], F32, tag="acc6" + sfx, name="acc6")
                giv = _ap(gi_sb, t0 * 2, [(GP, 3), (1, tc2)])
                acc6v = _ap(acc6, 0, [(tc2, 3), (1, tc2)])
                e1.tensor_tensor(acc6v, hw6v, giv, op=ALU.add)
                sig = pic.tile([128, 3 * tc2], F32, tag="sig" + sfx, name="sig")
                nc.scalar.activation(out=sig, in_=acc6, func=AF.Sigmoid)
                hw2 = pic.tile([128, tc2], F32, tag="hw2" + sfx, name="hw2")
                nc.scalar.activation(out=hw2, in_=hprev2, func=AF.Identity,
                                     bias=b2s, scale=w2s)
                rn = pic.tile([128, tc2], F32, tag="rn" + sfx, name="rn")
                e2.tensor_tensor(rn, sig[:, 0:tc2], hw2, op=ALU.mult)
                npre = pic.tile([128, tc2], F32, tag="npre" + sfx, name="npre")
                gnv = _ap(gi_sb, 3 * GP + t0 * 2, [(1, tc2)])
                e2.tensor_tensor(npre, rn, gnv, op=ALU.add)
                ntn = pic.tile([128, tc2], F32, tag="ntn" + sfx, name="ntn")
                nc.scalar.activation(out=ntn, in_=npre, func=AF.Tanh)
                bsc = pic.tile([128, tc2], F32, tag="bsc" + sfx, name="bsc")
                nc.vector.tensor_tensor(bsc, ntn, sig[:, 2 * tc2:3 * tc2],
                                        op=ALU.mult)
                for half in range(2):
                    nc.vector.tensor_tensor_scan(
                        _ap(h_sb, (t0 + 1) * 2 + half, [(2, CHUNK)]),
                        _ap(sig, tc2 + half, [(2, CHUNK)]),
                        _ap(bsc, half, [(2, CHUNK)]),
                        _ap(h_sb, t0 * 2 + half, [(1, 1)]),
                        op0=ALU.mult, op1=ALU.add,
                    )

            # ---- main pipeline ----
            # chunk c is ready after block 4c+3; spread its ITERS iterations
            # two per block starting at block 4c+4; last chunk is the tail
            sched = {b: [] for b in range(NBLK)}
            for c in range(NCHUNK - 1):
                for k in range(ITERS):
                    sched[4 * c + 4 + k // 2].append(c)
            for blk in range(NBLK):
                produce_block(blk)
                for c in sched[blk]:
                    picard_iter(c)
            for _ in range(ITERS - 1):
                picard_iter(NCHUNK - 1, sfx="T")

            # ---- batch sum over partitions ----
            sum_ps = sump.tile([1, 2 * T], F32)
            nc.tensor.matmul(sum_ps, ones1, h_sb[:, 2:2 * (T + 1)],
                             start=True, stop=True)
            sum_sb = scan.tile([1, 2 * T], F32)
            nc.vector.tensor_copy(sum_sb, sum_ps)
            nc.sync.dma_start(out=out[:], in_=sum_sb)

    _strip_same_engine_waits(nc)
    return nc


_ENG_PFX = {
    mybir.EngineType.Activation: "Activation",
    mybir.EngineType.DVE: "DVE",
    mybir.EngineType.PE: "PE",
    mybir.EngineType.Pool: "Pool",
    mybir.EngineType.SP: "SP",
}


def _strip_same_engine_waits(nc):
    """Compute-engine instruction formats have a single sync-wait slot; drop
    vacuous same-engine waits and transitively-covered DMA waits, and split
    the kernel-tail multi-wait drains (single-wait CTRL struct)."""
    multi = []
    for inst in nc.inst_map.values():
        si = inst.sync_info
        if not si or not si.on_wait or len(si.on_wait) <= 1:
            continue
        pfx = _ENG_PFX.get(inst.engine)
        if pfx is not None:
            kept = [
                w for w in si.on_wait
                if not (w.ant_name or "").startswith(pfx + "_")
            ]
            if len(kept) != len(si.on_wait):
                si.on_wait = kept
        if len(si.on_wait) > 1 and type(inst).__name__ == "InstDMACopy":
            comp = [
                w for w in si.on_wait
                if not (w.ant_name or "").startswith(("DMAHW", "DMASW"))
            ]
            if comp:
                si.on_wait = comp
        if len(si.on_wait) > 1:
            multi.append((inst.name, type(inst).__name__, str(inst.engine),
                          [w.ant_name for w in si.on_wait]))

    for block in nc.m.functions[0].blocks:
        insts = block.instructions
        for idx in range(len(insts) - 1, -1, -1):
            inst = insts[idx]
            si = inst.sync_info
            if (
                type(inst).__name__ != "InstDrain"
                or not si or not si.on_wait or len(si.on_wait) <= 1
            ):
                continue
            waits = list(si.on_wait)
            si.on_wait = waits[-1:]
            pre = []
            for k, w in enumerate(waits[:-1]):
                d = mybir.InstDrain(name=f"{inst.name}-w{k}", ins=[], outs=[])
                d.engine = inst.engine
                d.sync_info = mybir.SyncInfo(on_wait=[w], on_update=[])
                pre.append(d)
            insts[idx:idx] = pre
            multi = [m for m in multi if m[0] != inst.name]

    if multi:
        import sys
        print(f"[kernel] WARNING: {len(multi)} instructions still have >1 "
              f"sync wait: {multi[:8]}", file=sys.stderr)


def kernel(x, h0, w_ih, w_hh, b_ih, b_hh):
    x = np.asarray(x, dtype=np.float32)
    h0 = np.asarray(h0, dtype=np.float32)
    w_ih = np.asarray(w_ih, dtype=np.float32)
    w_hh = np.asarray(w_hh, dtype=np.float32)
    b_ih = np.asarray(b_ih, dtype=np.float32)
    b_hh = np.asarray(b_hh, dtype=np.float32)

    w0, w1, w2 = (float(v) for v in w_hh[:, 0])

    if "nc" not in _CACHE:
        _CACHE["nc"] = build_nc()
    nc = _CACHE["nc"]

    # ---- weights: rows (r, z, -z, n), hi/lo fp8 split, zero-slotted ----
    w4 = np.stack([w_ih[0], w_ih[1], -w_ih[1], w_ih[2]]) * WSC   # [4, D]
    w_hi = w4.astype(ml_dtypes.float8_e4m3)
    w_lo = (w4 - w_hi.astype(np.float32)).astype(ml_dtypes.float8_e4m3)
    w8f = np.stack([w_hi, w_lo])               # [hl2, g4, D]
    # d = c*256 + i*128 + p
    wd = w8f.reshape(2, 4, 2, 2, 128)          # [hl, g, c, i, p]
    wm = wd.transpose(4, 2, 3, 0, 1).reshape(128, 2, 2, 8)  # [p, c, i, hl*g]
    w8a = np.zeros((128, 2, NPAIR, 2, 32), dtype=ml_dtypes.float8_e4m3)
    for s in range(NPAIR):
        w8a[:, :, s, :, 8 * s:8 * s + 8] = wm
    w8a = np.ascontiguousarray(w8a.reshape(128, -1))

    I4 = np.eye(4, dtype=np.float32)
    s32 = np.kron(np.eye(16, dtype=np.float32),
                  np.vstack([I4, I4])).astype(np.float32)   # [128, 64]

    bias4 = np.array(
        [b_ih[0] + b_hh[0], b_ih[1] + b_hh[1], -(b_ih[1] + b_hh[1]),
         b_ih[2]], dtype=np.float32)

    # ---- x: quantize once, then per-core layout ----
    xq = (x * XSC).astype(ml_dtypes.float8_e4m3)    # [B, T, D]

    in_maps = []
    for core in range(N_CORES):
        xs = xq[core * B_LOC:(core + 1) * B_LOC]    # [256, T, D]
        # [b, blk, pair, tq, c, i, p] -> [blk, p, pair, c, i, tq, b]
        x7 = xs.reshape(B_LOC, NBLK, NPAIR, 2, 2, 2, 128)
        x7 = x7.transpose(1, 6, 2, 4, 5, 3, 0)
        x8a = np.ascontiguousarray(x7.reshape(NBLK, 128, -1))

        h0c = h0[0, core * B_LOC:(core + 1) * B_LOC, 0]    # [256]
        h0t = np.ascontiguousarray(h0c.reshape(2, 128).T)  # [128, 2]
        cstc = np.zeros((128, 13), dtype=np.float32)
        cstc[:, 0:4] = bias4
        cstc[:, 4] = w0
        cstc[:, 5] = w1
        cstc[:, 6] = -w1
        cstc[:, 7:9] = h0t
        cstc[:, 9] = 1.0
        cstc[:, 10] = w2
        cstc[:, 11] = float(b_hh[2])
        # per-row bias for the [32,512] git eviction: rows (pair, hl, g),
        # hi rows carry the gate bias, lo rows 0
        b32 = np.zeros((4, 2, 4), dtype=np.float32)
        b32[:, 0, :] = bias4
        cstc[0:32, 12] = b32.reshape(32)
        in_maps.append({"x8": x8a, "w8": w8a, "s32": s32, "cst": cstc})

    res = run_bass_kernel_spmd(nc, in_maps, core_ids=list(range(N_CORES)))
    total = np.zeros((T,), dtype=np.float64)
    for r in res.results:
        o = r["out"].reshape(T, 2).astype(np.float64)   # (t, half)
        total += o.sum(axis=1)
    return (total / B).astype(np.float32)
